# revision 37
# baseline (speedup 1.0000x reference)
"""Batched GCN (microtubule dynamics model) on 8 Trainium2 NeuronCores.

Math: the reference's gather/scale/scatter message passing over a fixed
52-node graph is a dense linear operator on the node axis:
    agg[b] = A @ h[b],  A[i, j] = sum over edges (j->i, incl self-loops)
                                   of dinv[src] * dinv[dst]
and A commutes with the shared linear layer, so each GNN layer is
    x += relu((A @ x) @ W_l^T + b_l),  batched over B.

Device strategy (pure data parallel, 512 batch elems / core):
  - activations live in SBUF as [128 hid partitions, 26624 token cols]
    (token = b*52 + node), fp16 on-chip, fp32 PSUM for the matmuls
  - per layer, per 8-batch-elem group (832 cols):
      MM1: per 104-token chunk, a fused transpose+W matmul (stationary =
      X chunk, moving = W_l^T) -> h^T token-major in PSUM -> copy to
      SBUF -> MM2: node-mix matmuls (stationary = h^T chunk, moving =
      blockdiag(A^T, A^T)) -> agg hid-major in PSUM -> fused relu+bias
      into a staging tile -> accumulating SWDGE DMA adds the residual
      into X (keeps the adds off the compute engines)
  - the PSUM->SBUF drains are the serial bottleneck (only ACT+DVE can
    read PSUM); they are emitted in bank-sized halves and spread over
    both engines with a greedy cost balancer
  - all constants ship in two packed DMAs and the batch input DMA is
    issued first so the PE warmup keeps the p-state high
  - decoder: d2 bias is folded into the PSUM->SBUF drain (per-partition
    bias); output slices are packed 4-up in partitions (32k..32k+5) and
    written with [6,1024] DMAs
"""

import numpy as np

N_FIL, N_SUB = 13, 4
N_NODES = N_FIL * N_SUB          # 52
FEAT = 6
HID = 128
N_LAYERS = 3
BATCH = 4096
N_CORES = 8
B_C = BATCH // N_CORES           # 512 batch elems per core
TOK = B_C * N_NODES              # 26624 token columns per core
PAIR_T = 2 * N_NODES             # 104 tokens per chunk
GROUP_B = 16                     # batch elems per inner tile
GROUP_T = GROUP_B * N_NODES      # 832 token columns per inner tile
N_PAIRS = GROUP_B // 2           # 8 chunks of 104 tokens per group
N_GROUPS = B_C // GROUP_B        # 32 groups per layer
ESLICE = 1024                    # encoder/decoder column slice
N_ESLICES = TOK // ESLICE        # 26

# fp16 constant pack column offsets: winT, wgT(3), wd1T, wd2T, a2, zero
C_WIN = 0
C_WG = 128            # 3 * 128
C_WD1 = C_WG + 384
C_WD2 = C_WD1 + 128
C_A2 = C_WD2 + 8      # a2 104 cols (pad to 8-col alignment)
C_BD2B = C_A2 + 104
C_ZERO = C_BD2B + 384
PACK16_COLS = C_ZERO + 1024
# fp32 pack: bin, bg(3), bd1, bd2y
PACK32_COLS = 8

_CACHE = {}
_LAST_EXEC_NS = None
_LAST_TRACE = []


class _Balancer:
    """Greedy ACT/DVE picker for PSUM->SBUF drains: assign each op to the
    engine with the smallest projected finish time (cost-model ns)."""

    def __init__(self):
        self.load = {"act": 0.0, "dve": 0.0}

    def pick(self, width):
        # cost-model: ACT 0.8333 ns/col + 185 init; DVE 1.0417 + 125
        cost = {"act": width * 0.8333 + 185, "dve": width * 1.0417 + 125}
        eng = min(self.load, key=lambda e: self.load[e] + cost[e])
        self.load[eng] += cost[eng]
        return eng


def _build_nc():
    import concourse.bacc as bacc
    import concourse.mybir as mybir
    from concourse.tile import TileContext

    f32 = mybir.dt.float32
    f16 = mybir.dt.float16
    Alu = mybir.AluOpType
    Relu = mybir.ActivationFunctionType.Relu

    nc = bacc.Bacc(trn_type="TRN2", dynamic_dma_scratch_size=32768)

    qT_d = nc.dram_tensor("qT", [FEAT, TOK], f16, kind="ExternalInput")
    pk16_d = nc.dram_tensor("pk16", [128, PACK16_COLS], f16, kind="ExternalInput")
    pk32_d = nc.dram_tensor("pk32", [128, PACK32_COLS], f32, kind="ExternalInput")
    yt_d = nc.dram_tensor("yt", [TOK, FEAT], f16, kind="ExternalOutput")

    bal = _Balancer()

    with TileContext(nc) as tc:
        with (
            tc.sbuf_pool(name="const", bufs=1) as cp,
            tc.sbuf_pool(name="work", bufs=4) as wp,
        ):
            ps_cm = tc.psum_pool(name="ps", bufs=2)
            pp = ps_cm.__enter__()
            pk16 = cp.tile([128, PACK16_COLS], f16)
            nc.sync.dma_start(pk16, pk16_d[:, :])
            pk32 = cp.tile([128, PACK32_COLS], f32)
            nc.sync.dma_start(pk32, pk32_d[:, :])
            qT = cp.tile([FEAT, TOK], f16)
            QQ = TOK // 4
            for qi in range(4):
                nc.sync.dma_start(qT[:, qi * QQ:(qi + 1) * QQ],
                                  qT_d[:, qi * QQ:(qi + 1) * QQ])

            winT = pk16[:FEAT, C_WIN:C_WIN + 128]
            wd1T = pk16[:, C_WD1:C_WD1 + 128]
            wd2T = pk16[:, C_WD2:C_WD2 + FEAT]
            a2 = pk16[:PAIR_T, C_A2:C_A2 + PAIR_T]
            zero_s = pk16[:, C_ZERO:C_ZERO + 1024]
            bin_s = pk32[:, 0:1]
            bd1_s = pk32[:, 4:5]
            bd2b = pk16[:, C_BD2B:C_BD2B + 384]

            X = cp.tile([128, TOK], f16)

            def drain(out, in_, bias=None, relu=False, eng=None):
                """One PSUM->SBUF drain on the least-loaded engine."""
                w = 1
                for d in out.shape[1:]:
                    w *= d
                if eng is None:
                    eng = bal.pick(w)
                if relu:
                    if eng == "act":
                        nc.scalar.activation(out, in_, Relu, bias=bias)
                    else:
                        z = zero_s[:, :w]
                        if len(out.shape) == 3:
                            z = z.rearrange("q (c u) -> q c u",
                                            u=out.shape[-1])
                        nc.vector.scalar_tensor_tensor(
                            out, in_, bias, z,
                            op0=Alu.add, op1=Alu.max)
                elif bias is not None:
                    if eng == "act":
                        nc.scalar.add(out, in_, bias)
                    else:
                        nc.vector.tensor_scalar_add(out, in_, bias)
                else:
                    if eng == "act":
                        nc.scalar.copy(out, in_)
                    else:
                        nc.vector.tensor_copy(out, in_)

            # -------- PE p-state warmup (runs while qT DMA is in flight) --
            junk = wp.tile([128, 512], f16, bufs=1, name="junk")
            nc.gpsimd.memset(junk, 0.0)
            for wi in range(10):
                warm_ps = pp.tile([128, 512], f32, tag="ps_s", bufs=2,
                                  name="warm_ps")
                nc.tensor.matmul(warm_ps, junk[:, :128], junk,
                                 start=True, stop=True)

            # -------- encoder: X = relu(W_in @ q^T + b_in) ----------------
            for s in range(N_ESLICES):
                cols = slice(s * ESLICE, (s + 1) * ESLICE)
                enc_ps = pp.tile([128, ESLICE], f32, tag="ps_b", bufs=2,
                                 name="enc_ps")
                for hh in range(2):
                    nc.tensor.matmul(
                        enc_ps[:, hh * 512:(hh + 1) * 512], winT,
                        qT[:, s * ESLICE + hh * 512:
                           s * ESLICE + (hh + 1) * 512],
                        start=True, stop=True)
                drain(X[:, cols], enc_ps, bin_s, relu=True,
                      eng="act" if s % 2 == 0 else "dve")

            # -------- GNN layers: x += relu(A (x W_l^T) + b_l) -----------
            # MM1 fuses the transpose and the W matmul: stationary = X
            # chunk (104 token cols), moving = W_l^T, out = h^T token-major.
            for l in range(N_LAYERS):
                wgTl = pk16[:, C_WG + l * 128:C_WG + (l + 1) * 128]
                bgl = pk32[:, 1 + l:2 + l]
                for g in range(N_GROUPS):
                    c0 = g * GROUP_T

                    hts = wp.tile([128, 128 * N_PAIRS], f16, bufs=6,
                                  name="hts")
                    ht_ps = pp.tile([128, 128 * N_PAIRS], f32, tag="ps_s",
                                    bufs=2, name="ht_ps")
                    for p in range(N_PAIRS):
                        nc.tensor.matmul(
                            ht_ps[:PAIR_T, p * 128:(p + 1) * 128],
                            X[:, c0 + p * PAIR_T:c0 + (p + 1) * PAIR_T],
                            wgTl,
                            start=True, stop=True,
                        )
                    drain(hts[:PAIR_T, :], ht_ps[:PAIR_T, :], eng="act")

                    # node mix back to hid-major: agg[hid,(g,i)] =
                    #   sum_j h^T[(g,j), hid] * A[i,j]
                    # each chunk's output sits 128-aligned so no matmul
                    # write ever crosses a PSUM bank boundary
                    agg_ps = pp.tile([128, 128 * N_PAIRS], f32, tag="ps_b",
                                     bufs=2, name="agg_ps")
                    for p in range(N_PAIRS):
                        nc.tensor.matmul(
                            agg_ps[:, p * 128:p * 128 + PAIR_T],
                            hts[:PAIR_T, p * 128:(p + 1) * 128],
                            a2,
                            start=True, stop=True,
                        )

                    # t = relu(agg + b_l) into a 2-group staging tile; one
                    # accumulating SWDGE DMA per 2 groups does x += t
                    if g % 2 == 0:
                        r2 = wp.tile([128, 2 * GROUP_T], f16, bufs=12,
                                     name="r2")
                    rbase = (g % 2) * GROUP_T
                    agg_v = agg_ps.rearrange("q (c w) -> q c w", w=128)[:, :, :PAIR_T]
                    r2_v = r2[:, rbase:rbase + GROUP_T].rearrange(
                        "q (c w) -> q c w", w=PAIR_T)
                    drain(r2_v, agg_v, bgl, relu=True, eng="dve")
                    if g % 2 == 1:
                        nc.gpsimd.dma_start(
                            X[:, (g - 1) * GROUP_T:(g + 1) * GROUP_T], r2,
                            accum_op=Alu.add,
                        )

            ps_cm.__exit__(None, None, None)
            psd_cm = tc.psum_pool(name="psd", bufs=2)
            pp2 = psd_cm.__enter__()

            # -------- decoder --------------------------------------------
            # d1 = relu(W_d1 x + b_d1) in hid-major; d2 uses the fused
            # transpose trick (stationary = d1 chunk, moving = W_d2^T) so
            # each 128-token chunk costs only 6 moving columns and lands
            # token-major in PSUM: y^T[token, feat]. The d2 bias rides the
            # PSUM drain as a broadcast tensor_tensor add.
            N_CH = TOK // 128                            # 208 chunks
            WCH = 16                                     # chunks per y window
            y_ps = None
            y_cols = 0
            w_base = 0
            for j in range(N_ESLICES):
                cols = slice(j * ESLICE, (j + 1) * ESLICE)
                d1_ps = pp2.tile([128, ESLICE], f32, tag="d", bufs=3,
                                 name="d1_ps")
                for hh in range(2):
                    nc.tensor.matmul(
                        d1_ps[:, hh * 512:(hh + 1) * 512], wd1T,
                        X[:, j * ESLICE + hh * 512:
                          j * ESLICE + (hh + 1) * 512],
                        start=True, stop=True)
                d1s = wp.tile([128, ESLICE], f16, bufs=4, name="d1s")
                drain(d1s, d1_ps, bd1_s, relu=True,
                      eng="act" if j % 2 == 0 else "dve")
                for cc in range(8):
                    ch = j * 8 + cc                      # global 128-tok chunk
                    if y_ps is None:
                        w_base = ch
                        y_cols = min(WCH, N_CH - w_base) * FEAT
                        y_ps = pp2.tile([128, WCH * FEAT], f32, tag="y",
                                        bufs=2, name="y_ps")
                    o = (ch - w_base) * FEAT
                    nc.tensor.matmul(
                        y_ps[:, o:o + FEAT],
                        d1s[:, cc * 128:(cc + 1) * 128], wd2T,
                        start=True, stop=True,
                    )
                    if o + FEAT == y_cols:
                        y4s = wp.tile([128, WCH * FEAT], f16, bufs=4,
                                      name="y4s")
                        nc.vector.tensor_tensor(
                            y4s[:, :y_cols], y_ps[:, :y_cols],
                            bd2b[:, :y_cols], op=Alu.add)
                        nch = y_cols // FEAT
                        dma_eng = nc.sync if (w_base // WCH) % 2 == 0 else nc.gpsimd
                        dma_eng.dma_start(
                            yt_d.rearrange("(c p) f -> p c f", p=128)
                                [:, w_base:w_base + nch, :],
                            y4s[:, :y_cols].rearrange(
                                "p (c f) -> p c f", f=FEAT),
                        )
                        y_ps = None
            psd_cm.__exit__(None, None, None)

    nc.finalize()
    return nc


def _host_prep(inputs):
    q = np.asarray(inputs["q_current"], np.float32).reshape(BATCH, N_NODES, FEAT)
    W_in = np.asarray(inputs["W_in"], np.float32)
    b_in = np.asarray(inputs["b_in"], np.float32)
    W_gnn = np.asarray(inputs["W_gnn"], np.float32)
    b_gnn = np.asarray(inputs["b_gnn"], np.float32)
    W_d1 = np.asarray(inputs["W_d1"], np.float32)
    b_d1 = np.asarray(inputs["b_d1"], np.float32)
    W_d2 = np.asarray(inputs["W_d2"], np.float32)
    b_d2 = np.asarray(inputs["b_d2"], np.float32)
    edge = np.asarray(inputs["edge_index"]).astype(np.int64)

    # dense normalized adjacency (PyG GCNConv default w/ self-loops)
    loops = np.arange(N_NODES, dtype=np.int64)
    src = np.concatenate([edge[0], loops])
    dst = np.concatenate([edge[1], loops])
    deg = np.zeros(N_NODES, np.float32)
    np.add.at(deg, dst, 1.0)
    dinv = 1.0 / np.sqrt(np.maximum(deg, 1e-12))
    A = np.zeros((N_NODES, N_NODES), np.float32)
    np.add.at(A, (dst, src), dinv[src] * dinv[dst])

    a2 = np.zeros((PAIR_T, PAIR_T), np.float32)
    a2[:N_NODES, :N_NODES] = A.T
    a2[N_NODES:, N_NODES:] = A.T

    pk16 = np.zeros((128, PACK16_COLS), np.float16)
    pk16[:FEAT, C_WIN:C_WIN + 128] = W_in.T.astype(np.float16)
    wgT = W_gnn.transpose(2, 0, 1).astype(np.float16)   # [HID, 3, HID]
    for l in range(N_LAYERS):
        pk16[:, C_WG + l * 128:C_WG + (l + 1) * 128] = wgT[:, l, :]
    pk16[:, C_WD1:C_WD1 + 128] = W_d1.T.astype(np.float16)
    pk16[:, C_WD2:C_WD2 + FEAT] = W_d2.T.astype(np.float16)
    pk16[:PAIR_T, C_A2:C_A2 + PAIR_T] = a2.astype(np.float16)
    # zero_s region stays zero

    pk32 = np.zeros((128, PACK32_COLS), np.float32)
    pk32[:, 0] = b_in
    pk32[:, 1:4] = b_gnn.T
    pk32[:, 4] = b_d1
    pk16[:, C_BD2B:C_BD2B + 384] = np.tile(b_d2.astype(np.float16), 64)[None, :]

    # per-core feature-major input [6, TOK], fp16
    q_flat = q.reshape(N_CORES, B_C * N_NODES, FEAT)
    qTs = [
        np.ascontiguousarray(q_flat[c].T).astype(np.float16)
        for c in range(N_CORES)
    ]
    return {"pk16": pk16, "pk32": pk32}, qTs


def kernel(**inputs):
    const, qTs = _host_prep(inputs)

    if "nc" not in _CACHE:
        _CACHE["nc"] = _build_nc()
    nc = _CACHE["nc"]

    from concourse.bass_utils import run_bass_kernel_spmd

    in_maps = [dict(const, qT=qTs[c]) for c in range(N_CORES)]
    res = run_bass_kernel_spmd(nc, in_maps, core_ids=list(range(N_CORES)))
    global _LAST_EXEC_NS
    _LAST_EXEC_NS = res.exec_time_ns
    if res.instructions_and_trace is not None:
        _LAST_TRACE.append(res.instructions_and_trace[1])

    outs = []
    for c in range(N_CORES):
        yt = res.results[c]["yt"]  # [TOK, 6]
        outs.append(np.asarray(yt, np.float32))
    y = np.concatenate(outs, axis=0)  # [BATCH*52, 6]
    return np.ascontiguousarray(y).reshape(BATCH, N_FIL, N_SUB, FEAT)
